# revision 29
# baseline (speedup 1.0000x reference)
"""Trainium2 Bass kernel for nn_BaseModel_74526272520550 (gnn_message_passing).

DeepH-style Hamiltonian construction. The whole network reduces, per edge e with
inverse partner e', to

    ham_{t1,t2}[g2b[e]] = 0.5 * (A12^T f_aug[e] + B12^T f_aug[e'])

where A12/B12 are [100, OS1*OS2] constants folded from the CG tensors, and
f_aug = feature_edge (+ feature_node[n1] on self edges). The partner row for the
(t2,t1) block is the transpose of this one, so each unordered inverse pair needs
exactly ONE device-computed row; transposition and row placement are pure index
permutations done during unsharding.

Device work per core: stream feature-major [100, C] arrays, 2 (or 4) float32r
matmuls per 512-item tile against the constant maps, PSUM -> SBUF -> DRAM.
Host work: index gathers / permutations only (the sharding + unsharding).
"""

import numpy as np

NUM_CORES = 8
TILE_N = 512        # items per matmul / PSUM tile
CHUNK = 2048        # items per DMA load (8KB per partition row)
PAD = 128           # per-core item-count padding granularity

COMMON_AM = [0, 0, 1, 2]
ORBITAL_NUM = [[1, 1], [2, 1, 1]]
ORBITAL_SUM = [4, 10]

STREAM_DT = "float32r"   # "float32r" (1 cyc/row, reduced precision) or "float32"

LAST_RESULT = None       # BassKernelResults of the most recent device run


# ---------------------------------------------------------------------------
# constant-map construction (pure numpy, tiny)
# ---------------------------------------------------------------------------

def _block_meta():
    offs, sizes = [], []
    off = 0
    for l1 in COMMON_AM:
        for l2 in COMMON_AM:
            sz = (2 * l1 + 1) * (2 * l2 + 1)
            offs.append(off)
            sizes.append(sz)
            off += sz
    return offs, sizes


_FOFF, _FSZ = _block_meta()


def _used_slots(t):
    nl = list(ORBITAL_NUM[t])
    out, roff = [], 0
    for i, l in enumerate(COMMON_AM):
        if len(nl) < l + 1 or nl[l] == 0:
            continue
        nl[l] -= 1
        out.append((i, l, roff))
        roff += 2 * l + 1
    return out


def _build_A(cg, t1, t2):
    """A in R^[100, OS1*OS2] with ham_row_vec = f @ A (row-major flatten)."""
    OS1, OS2 = ORBITAL_SUM[t1], ORBITAL_SUM[t2]
    A = np.zeros((100, OS1 * OS2), np.float32)
    for (i1, l1, roff) in _used_slots(t1):
        for (i2, l2, coff) in _used_slots(t2):
            boff = 4 * i1 + i2
            c = np.asarray(cg[(l1, l2)], np.float32)
            foff = _FOFF[boff]
            for i in range(2 * l1 + 1):
                for j in range(2 * l2 + 1):
                    A[foff:foff + _FSZ[boff], (roff + i) * OS2 + (coff + j)] = c[i, j, :]
    return A


def _swap_cols(t1, t2):
    """perm p with B12 = A21[:, p]: B12[:, r*OS2+c] = A21[:, c*OS1+r]."""
    OS1, OS2 = ORBITAL_SUM[t1], ORBITAL_SUM[t2]
    return np.arange(OS2 * OS1).reshape(OS2, OS1).T.reshape(-1)


# ---------------------------------------------------------------------------
# bass program builder (one program shared by all 8 cores)
# ---------------------------------------------------------------------------

_NC_CACHE = {}


def _mat_layout(spec_key):
    """Column offsets of each (group, stream) matrix in the packed const array."""
    offs, off = {}, 0
    for (name, rc, ns, Cg) in spec_key:
        for j in range(ns):
            offs[(name, j)] = off
            off += rc
    return offs, off


NBANKS = 8    # PSUM banks rotated by subtile


def _bufs(max_ns):
    """(NBUF, OBUF); 4-stream groups need smaller ld footprints."""
    return (2, 3) if max_ns > 2 else (4, 3)


def _plan(spec_key):
    """Flatten the group/chunk/subtile structure into static lists."""
    chunks = []      # (name, rc, ns, c0, cw, sub_start, n_subs)
    subs = []        # (chunk_idx, rc, j0, w, off)
    for (name, rc, ns, Cg) in spec_key:
        for c0 in range(0, Cg, CHUNK):
            cw = min(CHUNK, Cg - c0)
            ci = len(chunks)
            sub_start = len(subs)
            for j0 in range(0, cw, TILE_N):
                w = min(TILE_N, cw - j0)
                off = (len(subs) % NBANKS) * TILE_N
                subs.append((ci, rc, j0, w, off))
            chunks.append((name, rc, ns, c0, cw, sub_start,
                           len(subs) - sub_start))
    return chunks, subs


def _build_nc(spec_key):
    """Raw-bass SPMD program (one per kernel invocation, shared by all cores).

    This walrus build supports at most ONE semaphore wait per instruction
    (64B ISA structs have a single EVENTS slot and no wait splitting), which
    Tile's auto-generated sync routinely exceeds. So sync is hand-placed:
    wait_ge lowers to its own single-wait EventSemaphore instruction.

    Engine roles: SP = stream/const loads (one HWDGE ring, FIFO, one counting
    sem), ACT = output stores (second HWDGE ring), PE = matmuls, DVE =
    PSUM->SBUF copies. PSUM is one persistent [128, 8*512] region with banks
    rotated per subtile.
    """
    if spec_key in _NC_CACHE:
        return _NC_CACHE[spec_key]

    from contextlib import ExitStack

    import concourse.bass as bass
    import concourse.mybir as mybir

    sdt = getattr(mybir.dt, STREAM_DT)
    f32 = mybir.dt.float32

    moffs, mtot = _mat_layout(spec_key)
    chunks, subs = _plan(spec_key)
    C, S = len(chunks), len(subs)
    max_ns = max(ns for (_, _, ns, _) in spec_key)
    NBUF, OBUF = _bufs(max_ns)

    nc = bass.Bass()
    hs = {'mats_all': nc.declare_dram_parameter('mats_all', [100, mtot], sdt,
                                                isOutput=False)}
    for (name, rc, ns, Cg) in spec_key:
        nch = -(-Cg // CHUNK)
        for j in range(ns):
            # chunk-major layout: each full chunk is one contiguous 100*CHUNK
            # block, so a load is a single fully-sequential HBM read
            hs[f"{name}_s{j}"] = nc.declare_dram_parameter(
                f"{name}_s{j}", [nch, 100, CHUNK], sdt, isOutput=False)
        hs[f"{name}_y"] = nc.declare_dram_parameter(
            f"{name}_y", [rc, Cg], f32, isOutput=True)

    ctx = ExitStack()
    mats = ctx.enter_context(nc.sbuf_tensor("mats", [100, mtot], sdt))
    ld = [[ctx.enter_context(
        nc.sbuf_tensor(f"ld{j}_{b}", [100, CHUNK], sdt))
        for b in range(NBUF)] for j in range(max_ns)]
    ot = [ctx.enter_context(
        nc.sbuf_tensor(f"ot_{b}", [128, CHUNK], f32))
        for b in range(OBUF)]
    ps = ctx.enter_context(
        nc.psum_tensor("ps", [128, NBANKS * TILE_N], f32))
    # per-buffer-slot DMA semaphores: successive DMAs on one sem are strictly
    # serialized by the consume/free protocol, so out-of-order HWDGE
    # completion between different buffers can't confuse a waiter
    sem_mats = ctx.enter_context(nc.semaphore("sem_mats"))
    sem_ld = [[ctx.enter_context(nc.semaphore(f"sem_ld{j}_{b}"))
               for b in range(NBUF)] for j in range(max_ns)]
    sem_ot = [ctx.enter_context(nc.semaphore(f"sem_ot{b}"))
              for b in range(OBUF)]
    sem_pe = ctx.enter_context(nc.semaphore("sem_pe"))
    sem_dve = ctx.enter_context(nc.semaphore("sem_dve"))

    # SDMA engines are direction-bound: loads ride the 10-engine ingress set,
    # stores the 6-engine egress set, REGARDLESS of issuing ring. Multiple
    # load queues only add per-engine packet-switching overhead (measured
    # 21.6 -> 16.9 GB/s per engine), so: ALL loads on the sync ring, ALL
    # stores on gpsimd, which also keeps each DMA semaphore single-updater.
    ENGS = ('sync', 'scalar', 'gpsimd')

    def _in_issuer(j, b):
        return 'sync'

    def _out_issuer(c):
        return 'gpsimd'

    with ctx, nc.Block() as block:

        def _stream(eng, ename):
            if ename == 'sync':
                eng.dma_start(out=mats[:, :],
                              in_=hs['mats_all'][:, :]).then_inc(sem_mats, 16)

            def _store(x):
                (name, rc, ns, c0, cw, ss, nsub) = chunks[x]
                eng.wait_ge(sem_dve, ss + nsub)
                if x >= OBUF:
                    eng.wait_ge(sem_ot[x % OBUF], 16 * (x // OBUF))
                eng.dma_start(
                    out=hs[f"{name}_y"][:, c0:c0 + cw],
                    in_=ot[x % OBUF][:rc, :cw],
                ).then_inc(sem_ot[x % OBUF], 16)

            for c, (name, rc, ns, c0, cw, ss, nsub) in enumerate(chunks):
                b = c % NBUF
                r = c // NBUF
                mine = [j for j in range(ns) if _in_issuer(j, b) == ename]
                if mine and c >= NBUF:
                    # buffer slot free once PE finished chunk c-NBUF
                    pc = chunks[c - NBUF]
                    eng.wait_ge(sem_pe, pc[5] + pc[6])
                for j in mine:
                    if r > 0:
                        # trivially satisfied (consumer progress implies the
                        # previous DMA on this sem completed); proves update
                        # ordering to the race detector
                        eng.wait_ge(sem_ld[j][b], 16 * r)
                    eng.dma_start(
                        out=ld[j][b][:, :cw],
                        in_=hs[f"{name}_s{j}"][c0 // CHUNK, :, :cw],
                    ).then_inc(sem_ld[j][b], 16)
                # output stores lag 2 chunks so their compute-wait never
                # starves this stream's load lookahead
                if c >= 2 and _out_issuer(c - 2) == ename:
                    _store(c - 2)
            for x in range(max(C - 2, 0), C):
                if _out_issuer(x) == ename:
                    _store(x)
            for b in range(OBUF):
                if _out_issuer(b) != ename:
                    continue
                n_uses = len([1 for c in range(C) if c % OBUF == b])
                if n_uses:
                    eng.wait_ge(sem_ot[b], 16 * n_uses)

        @block.sync
        def _(sync):
            _stream(sync, 'sync')

        @block.scalar
        def _(scalar):
            _stream(scalar, 'scalar')

        @block.gpsimd
        def _(gpsimd):
            _stream(gpsimd, 'gpsimd')

        @block.tensor
        def _(tensor):
            for k, (ci, rc, j0, w, off) in enumerate(subs):
                (name, _, ns, c0, cw, ss, nsub) = chunks[ci]
                b = ci % NBUF
                r = ci // NBUF
                if k == 0:
                    tensor.wait_ge(sem_mats, 16)
                if j0 == 0:
                    for j in range(ns):
                        tensor.wait_ge(sem_ld[j][b], 16 * (r + 1))
                if k >= NBANKS:
                    tensor.wait_ge(sem_dve, k - NBANKS + 1)
                for j in range(ns):
                    mo = moffs[(name, j)]
                    mm = tensor.matmul(
                        ps[:rc, off:off + w],
                        lhsT=mats[:, mo:mo + rc],
                        rhs=ld[j][b][:, j0:j0 + w],
                        start=(j == 0),
                        stop=(j == ns - 1),
                        skip_group_check=True,
                    )
                mm.then_inc(sem_pe, 1)

        @block.vector
        def _(vector):
            for k, (ci, rc, j0, w, off) in enumerate(subs):
                (name, _, ns, c0, cw, ss, nsub) = chunks[ci]
                if j0 == 0 and ci >= OBUF:
                    # out slot free once its previous out-DMA completed
                    vector.wait_ge(sem_ot[ci % OBUF], 16 * (ci // OBUF))
                vector.wait_ge(sem_pe, k + 1)
                vector.tensor_copy(
                    out=ot[ci % OBUF][:rc, j0:j0 + w],
                    in_=ps[:rc, off:off + w],
                ).then_inc(sem_dve, 1)

    _NC_CACHE[spec_key] = nc
    return nc


# ---------------------------------------------------------------------------
# host-side orchestration
# ---------------------------------------------------------------------------

def _pad_up(n, m):
    return max(((n + m - 1) // m) * m, m)


def kernel(**inputs):
    from concourse.bass_utils import run_bass_kernel_spmd

    fe = np.asarray(inputs['feature_edge'], dtype=np.float32)
    fn = np.asarray(inputs['feature_node'], dtype=np.float32)
    S = np.asarray(inputs['S_hop'], dtype=np.float32)
    eih = np.asarray(inputs['edge_index_hop'])
    inv = np.asarray(inputs['edge_inverse']).astype(np.int64)
    atom_type = np.asarray(inputs['AtomType'])
    sel = {(0, 0): np.asarray(inputs['edge_sel_00']).astype(np.int64),
           (0, 1): np.asarray(inputs['edge_sel_01']).astype(np.int64),
           (1, 0): np.asarray(inputs['edge_sel_10']).astype(np.int64),
           (1, 1): np.asarray(inputs['edge_sel_11']).astype(np.int64)}
    cg = {(l1, l2): np.asarray(inputs[f'cg_{l1}_{l2}'], dtype=np.float32)
          for l1 in range(3) for l2 in range(3)}

    n1 = eih[0].astype(np.int64)
    n2 = eih[1].astype(np.int64)
    E = fe.shape[0]
    N = fn.shape[0]
    is_self = (n1 == n2) & (np.linalg.norm(S, axis=-1) < 1e-6)

    # g2b exactly as the reference computes it (sequential scatter, last wins)
    g2b = np.zeros(E, np.int64)
    for p in [(0, 0), (0, 1), (1, 0), (1, 1)]:
        g2b[sel[p]] = np.arange(len(sel[p]))

    # constant maps
    A = {(t1, t2): _build_A(cg, t1, t2) for t1 in range(2) for t2 in range(2)}
    B = {(t1, t2): A[(t2, t1)][:, _swap_cols(t1, t2)]
         for t1 in range(2) for t2 in range(2)}

    # feature-major copies for contiguous column gathers; fnz has a zero col at N
    feT = np.ascontiguousarray(fe.T)
    fnT = np.ascontiguousarray(fn.T)
    fnzT = np.concatenate([fnT, np.zeros((100, 1), np.float32)], axis=1)

    # ---- invariant check: structured (paired) fast path applicable? ----
    structured = _check_structured(E, inv, is_self, sel, atom_type, n1, n2)

    groups, writes, post_ham10 = _make_groups(
        structured, fe, fn, sel, inv, is_self, g2b, n1, A, B, E, N)
    # big-output groups first so the final store tail is small
    groups.sort(key=lambda g: -g['rc'])

    # ---- shard each group across cores, build in_maps ----
    spec_key = []
    meta = []
    for g in groups:
        n_items = len(g['streams'][0][1])
        per = -(-n_items // NUM_CORES)
        Cg = _pad_up(per, PAD)
        spec_key.append((g['name'], g['rc'], len(g['streams']), Cg))
        meta.append((g, n_items, per, Cg))
    spec_key = tuple(spec_key)

    moffs, mtot = _mat_layout(spec_key)
    mats_all = np.zeros((100, mtot), np.float32)
    for g in groups:
        for j in range(len(g['streams'])):
            off = moffs[(g['name'], j)]
            mats_all[:, off:off + g['rc']] = g['mats'][j]

    in_maps = [dict() for _ in range(NUM_CORES)]
    for c in range(NUM_CORES):
        in_maps[c]['mats_all'] = mats_all
    for (g, n_items, per, Cg) in meta:
        nch = -(-Cg // CHUNK)
        for j, (kind, idx) in enumerate(g['streams']):
            src = {'fe': feT, 'fn': fnT, 'fnz': fnzT}[kind]
            for c in range(NUM_CORES):
                lo = min(c * per, n_items)
                hi = min(lo + per, n_items)
                arr = np.zeros((100, nch * CHUNK), np.float32)
                if hi > lo:
                    arr[:, :hi - lo] = src[:, idx[lo:hi]]
                in_maps[c][f"{g['name']}_s{j}"] = np.ascontiguousarray(
                    arr.reshape(100, nch, CHUNK).transpose(1, 0, 2))

    nc = _build_nc(spec_key)
    global LAST_RESULT
    res = run_bass_kernel_spmd(nc, in_maps, core_ids=list(range(NUM_CORES)))
    LAST_RESULT = res
    results = res.results

    # ---- unshard: per group concat the per-core y slices ----
    y_of = {}
    for (g, n_items, per, Cg) in meta:
        parts = []
        for c in range(NUM_CORES):
            lo = min(c * per, n_items)
            hi = min(lo + per, n_items)
            if hi > lo:
                parts.append(np.asarray(results[c][f"{g['name']}_y"])[:, :hi - lo])
        y = np.concatenate(parts, axis=1) if parts else np.zeros((g['rc'], 0), np.float32)
        y_of[g['name']] = y.T          # [n_items, rc]

    # ---- assemble hams (index permutations only) ----
    shapes = [(len(sel[(0, 0)]), 4, 4), (len(sel[(0, 1)]), 4, 10),
              (len(sel[(1, 0)]), 10, 4), (len(sel[(1, 1)]), 10, 10)]
    hams = [np.zeros(s, np.float32) for s in shapes]
    for (gname, ham_idx, rows, transposed) in writes:
        r, c = shapes[ham_idx][1], shapes[ham_idx][2]
        y = y_of[gname]
        if transposed:
            blk = y.reshape(-1, c, r).transpose(0, 2, 1)
        else:
            blk = y.reshape(-1, r, c)
        hams[ham_idx][rows] = blk

    if post_ham10 is not None:
        # general path: ham10 = swap(ham01[eb12]) with jnp-style clamped gather
        eb12 = post_ham10
        hams[2] = hams[1][eb12].transpose(0, 2, 1)

    g2b_out = g2b.astype(np.int32)
    return (*hams, g2b_out)


def _check_structured(E, inv, is_self, sel, atom_type, n1, n2):
    if inv.min() < 0 or inv.max() >= E:
        return False
    if not np.array_equal(inv[inv], np.arange(E)):
        return False
    if not np.array_equal(inv == np.arange(E), is_self):
        return False
    allsel = np.concatenate([sel[p] for p in [(0, 0), (0, 1), (1, 0), (1, 1)]])
    if len(allsel) != E:
        return False
    if not np.array_equal(np.sort(allsel), np.arange(E)):
        return False
    t1 = atom_type[n1]
    t2 = atom_type[n2]
    for (a, b), s in sel.items():
        if not (np.all(t1[s] == a) and np.all(t2[s] == b)):
            return False
    # inverse edges swap the type pair
    if not (np.all(np.isin(inv[sel[(0, 1)]], sel[(1, 0)], assume_unique=True))):
        return False
    # self edges only on the diagonal (implied by type consistency + n1==n2)
    return True


def _make_groups(structured, fe, fn, sel, inv, is_self, g2b, n1, A, B, E, N):
    groups = []
    writes = []       # (group_name, ham_idx, row_indices, transposed)
    post_ham10 = None

    if structured:
        # off-diagonal: one item per sel01 edge; partner row of ham10 is its swap
        a = sel[(0, 1)]
        b = inv[a]
        groups.append(dict(name='p01', rc=40,
                           mats=[0.5 * A[(0, 1)], 0.5 * B[(0, 1)]],
                           streams=[('fe', a), ('fe', b)]))
        writes.append(('p01', 1, np.arange(len(a)), False))
        writes.append(('p01', 2, g2b[b], True))

        for t in (0, 1):
            s_tt = sel[(t, t)]
            hi = t * 2 + t      # 0 or 3
            rc = ORBITAL_SUM[t] ** 2
            selfm = is_self[s_tt]
            reg = s_tt[~selfm]
            can = reg[reg < inv[reg]]
            bp = inv[can]
            if len(can):
                groups.append(dict(name=f'p{t}{t}', rc=rc,
                                   mats=[0.5 * A[(t, t)], 0.5 * B[(t, t)]],
                                   streams=[('fe', can), ('fe', bp)]))
                writes.append((f'p{t}{t}', hi, g2b[can], False))
                writes.append((f'p{t}{t}', hi, g2b[bp], True))
            se = s_tt[selfm]
            if len(se):
                Msym = 0.5 * (A[(t, t)] + B[(t, t)])
                groups.append(dict(name=f's{t}', rc=rc,
                                   mats=[Msym, Msym],
                                   streams=[('fe', se), ('fn', n1[se])]))
                writes.append((f's{t}', hi, g2b[se], False))
    else:
        # general path: emulate the reference's clamped gathers row by row
        def partner(s12, p21):
            s21 = sel[p21]
            j = np.clip(g2b[np.clip(inv[s12], 0, E - 1)], 0, max(len(s21) - 1, 0))
            return s21[j] if len(s21) else np.zeros(len(s12), np.int64)

        for (pair, hi) in [((0, 0), 0), ((0, 1), 1), ((1, 1), 3)]:
            t1t, t2t = pair
            s12 = sel[pair]
            if not len(s12):
                continue
            b = partner(s12, (t2t, t1t))
            plain = ~is_self[s12] & ~is_self[b]
            rc = ORBITAL_SUM[t1t] * ORBITAL_SUM[t2t]
            nm = f'g{t1t}{t2t}'
            idx_p = np.nonzero(plain)[0]
            idx_m = np.nonzero(~plain)[0]
            if len(idx_p):
                groups.append(dict(name=nm, rc=rc,
                                   mats=[0.5 * A[pair], 0.5 * B[pair]],
                                   streams=[('fe', s12[idx_p]), ('fe', b[idx_p])]))
                writes.append((nm, hi, idx_p, False))
            if len(idx_m):
                am, bm = s12[idx_m], b[idx_m]
                fa = np.where(is_self[am], n1[am], N)
                fb = np.where(is_self[bm], n1[bm], N)
                groups.append(dict(name=nm + 'x', rc=rc,
                                   mats=[0.5 * A[pair], 0.5 * A[pair],
                                         0.5 * B[pair], 0.5 * B[pair]],
                                   streams=[('fe', am), ('fnz', fa),
                                            ('fe', bm), ('fnz', fb)]))
                writes.append((nm + 'x', hi, idx_m, False))
        s10 = sel[(1, 0)]
        n01 = len(sel[(0, 1)])
        post_ham10 = np.clip(g2b[np.clip(inv[s10], 0, E - 1)], 0, max(n01 - 1, 0))

    return groups, writes, post_ham10


if __name__ == '__main__':
    pass


# revision 32
# speedup vs baseline: 1.1544x; 1.1544x over previous
"""Trainium2 Bass kernel for nn_BaseModel_74526272520550 (gnn_message_passing).

DeepH-style Hamiltonian construction. The whole network reduces, per edge e with
inverse partner e', to

    ham_{t1,t2}[g2b[e]] = 0.5 * (A12^T f_aug[e] + B12^T f_aug[e'])

where A12/B12 are [100, OS1*OS2] constants folded from the CG tensors, and
f_aug = feature_edge (+ feature_node[n1] on self edges). The partner row for the
(t2,t1) block is the transpose of this one, so each unordered inverse pair needs
exactly ONE device-computed row; transposition and row placement are pure index
permutations done during unsharding.

Device work per core: stream feature-major [100, C] arrays, 2 (or 4) float32r
matmuls per 512-item tile against the constant maps, PSUM -> SBUF -> DRAM.
Host work: index gathers / permutations only (the sharding + unsharding).
"""

import numpy as np

NUM_CORES = 8
TILE_N = 512        # items per matmul / PSUM tile
CHUNK = 2048        # items per DMA load (8KB per partition row)
PAD = 128           # per-core item-count padding granularity

COMMON_AM = [0, 0, 1, 2]
ORBITAL_NUM = [[1, 1], [2, 1, 1]]
ORBITAL_SUM = [4, 10]

STREAM_DT = "float32r"   # "float32r" (1 cyc/row, reduced precision) or "float32"

LAST_RESULT = None       # BassKernelResults of the most recent device run


# ---------------------------------------------------------------------------
# constant-map construction (pure numpy, tiny)
# ---------------------------------------------------------------------------

def _block_meta():
    offs, sizes = [], []
    off = 0
    for l1 in COMMON_AM:
        for l2 in COMMON_AM:
            sz = (2 * l1 + 1) * (2 * l2 + 1)
            offs.append(off)
            sizes.append(sz)
            off += sz
    return offs, sizes


_FOFF, _FSZ = _block_meta()


def _used_slots(t):
    nl = list(ORBITAL_NUM[t])
    out, roff = [], 0
    for i, l in enumerate(COMMON_AM):
        if len(nl) < l + 1 or nl[l] == 0:
            continue
        nl[l] -= 1
        out.append((i, l, roff))
        roff += 2 * l + 1
    return out


def _build_A(cg, t1, t2):
    """A in R^[100, OS1*OS2] with ham_row_vec = f @ A (row-major flatten)."""
    OS1, OS2 = ORBITAL_SUM[t1], ORBITAL_SUM[t2]
    A = np.zeros((100, OS1 * OS2), np.float32)
    for (i1, l1, roff) in _used_slots(t1):
        for (i2, l2, coff) in _used_slots(t2):
            boff = 4 * i1 + i2
            c = np.asarray(cg[(l1, l2)], np.float32)
            foff = _FOFF[boff]
            for i in range(2 * l1 + 1):
                for j in range(2 * l2 + 1):
                    A[foff:foff + _FSZ[boff], (roff + i) * OS2 + (coff + j)] = c[i, j, :]
    return A


def _swap_cols(t1, t2):
    """perm p with B12 = A21[:, p]: B12[:, r*OS2+c] = A21[:, c*OS1+r]."""
    OS1, OS2 = ORBITAL_SUM[t1], ORBITAL_SUM[t2]
    return np.arange(OS2 * OS1).reshape(OS2, OS1).T.reshape(-1)


# ---------------------------------------------------------------------------
# bass program builder (one program shared by all 8 cores)
# ---------------------------------------------------------------------------

_NC_CACHE = {}


def _mat_layout(spec_key):
    """Column offsets of each (group, stream) matrix in the packed const array."""
    offs, off = {}, 0
    for (name, rc, ns, Cg) in spec_key:
        for j in range(ns):
            offs[(name, j)] = off
            off += rc
    return offs, off


NBANKS = 8    # PSUM banks rotated by subtile


def _bufs(max_ns):
    """(NBUF, OBUF); 4-stream groups need smaller ld footprints."""
    return (2, 3) if max_ns > 2 else (4, 3)


def _plan(spec_key):
    """Flatten the group/chunk/subtile structure into static lists."""
    chunks = []      # (name, rc, ns, c0, cw, sub_start, n_subs)
    subs = []        # (chunk_idx, rc, j0, w, off)
    for gi, (name, rc, ns, Cg) in enumerate(spec_key):
        # small leading chunks on the first group so PE starts ~10us sooner
        # (a full pipeline-fill of NBUF*CHUNK items gates the first matmul)
        bounds = [0, 512, 1024] if gi == 0 and Cg >= 2048 else [0]
        c0s = [b for b in bounds if b < Cg] + list(range(CHUNK, Cg, CHUNK))
        for i, c0 in enumerate(c0s):
            nxt = c0s[i + 1] if i + 1 < len(c0s) else Cg
            cw = nxt - c0
            ci = len(chunks)
            sub_start = len(subs)
            for j0 in range(0, cw, TILE_N):
                w = min(TILE_N, cw - j0)
                off = (len(subs) % NBANKS) * TILE_N
                subs.append((ci, rc, j0, w, off))
            chunks.append((name, rc, ns, c0, cw, sub_start,
                           len(subs) - sub_start))
    return chunks, subs


def _build_nc(spec_key):
    """Raw-bass SPMD program (one per kernel invocation, shared by all cores).

    This walrus build supports at most ONE semaphore wait per instruction
    (64B ISA structs have a single EVENTS slot and no wait splitting), which
    Tile's auto-generated sync routinely exceeds. So sync is hand-placed:
    wait_ge lowers to its own single-wait EventSemaphore instruction.

    Engine roles: SP = stream/const loads (one HWDGE ring, FIFO, one counting
    sem), ACT = output stores (second HWDGE ring), PE = matmuls, DVE =
    PSUM->SBUF copies. PSUM is one persistent [128, 8*512] region with banks
    rotated per subtile.
    """
    if spec_key in _NC_CACHE:
        return _NC_CACHE[spec_key]

    from contextlib import ExitStack

    import concourse.bass as bass
    import concourse.mybir as mybir

    sdt = getattr(mybir.dt, STREAM_DT)
    f32 = mybir.dt.float32

    moffs, mtot = _mat_layout(spec_key)
    chunks, subs = _plan(spec_key)
    C, S = len(chunks), len(subs)
    max_ns = max(ns for (_, _, ns, _) in spec_key)
    NBUF, OBUF = _bufs(max_ns)

    nc = bass.Bass()
    hs = {'mats_all': nc.declare_dram_parameter('mats_all', [100, mtot], sdt,
                                                isOutput=False)}
    for (name, rc, ns, Cg) in spec_key:
        nch = -(-Cg // CHUNK)
        for j in range(ns):
            # chunk-major layout: each full chunk is one contiguous 100*CHUNK
            # block, so a load is a single fully-sequential HBM read
            hs[f"{name}_s{j}"] = nc.declare_dram_parameter(
                f"{name}_s{j}", [nch, 100, CHUNK], sdt, isOutput=False)
        hs[f"{name}_y"] = nc.declare_dram_parameter(
            f"{name}_y", [rc, Cg], f32, isOutput=True)

    ctx = ExitStack()
    mats = ctx.enter_context(nc.sbuf_tensor("mats", [100, mtot], sdt))
    ld = [[ctx.enter_context(
        nc.sbuf_tensor(f"ld{j}_{b}", [100, CHUNK], sdt))
        for b in range(NBUF)] for j in range(max_ns)]
    ot = [ctx.enter_context(
        nc.sbuf_tensor(f"ot_{b}", [128, CHUNK], f32))
        for b in range(OBUF)]
    ps = ctx.enter_context(
        nc.psum_tensor("ps", [128, NBANKS * TILE_N], f32))
    # per-buffer-slot DMA semaphores: successive DMAs on one sem are strictly
    # serialized by the consume/free protocol, so out-of-order HWDGE
    # completion between different buffers can't confuse a waiter
    sem_mats = ctx.enter_context(nc.semaphore("sem_mats"))
    sem_ld = [[ctx.enter_context(nc.semaphore(f"sem_ld{j}_{b}"))
               for b in range(NBUF)] for j in range(max_ns)]
    sem_ot = [ctx.enter_context(nc.semaphore(f"sem_ot{b}"))
              for b in range(OBUF)]
    sem_pe = ctx.enter_context(nc.semaphore("sem_pe"))
    sem_dve = ctx.enter_context(nc.semaphore("sem_dve"))

    # SDMA engines are direction-bound: loads ride the 10-engine ingress set,
    # stores the 6-engine egress set, REGARDLESS of issuing ring. Multiple
    # load queues only add per-engine packet-switching overhead (measured
    # 21.6 -> 16.9 GB/s per engine), so: ALL loads on the sync ring, ALL
    # stores on gpsimd, which also keeps each DMA semaphore single-updater.
    ENGS = ('sync', 'scalar', 'gpsimd')

    def _in_issuer(j, b):
        return 'sync' if j % 2 == 0 else 'scalar'

    def _out_issuer(c):
        return 'gpsimd'

    with ctx, nc.Block() as block:

        def _stream(eng, ename):
            if ename == 'sync':
                eng.dma_start(out=mats[:, :],
                              in_=hs['mats_all'][:, :]).then_inc(sem_mats, 16)

            def _store(x):
                (name, rc, ns, c0, cw, ss, nsub) = chunks[x]
                eng.wait_ge(sem_dve, ss + nsub)
                if x >= OBUF:
                    eng.wait_ge(sem_ot[x % OBUF], 16 * (x // OBUF))
                eng.dma_start(
                    out=hs[f"{name}_y"][:, c0:c0 + cw],
                    in_=ot[x % OBUF][:rc, :cw],
                ).then_inc(sem_ot[x % OBUF], 16)

            for c, (name, rc, ns, c0, cw, ss, nsub) in enumerate(chunks):
                b = c % NBUF
                r = c // NBUF
                mine = [j for j in range(ns) if _in_issuer(j, b) == ename]
                if mine and c >= NBUF:
                    # buffer slot free once PE finished chunk c-NBUF
                    pc = chunks[c - NBUF]
                    eng.wait_ge(sem_pe, pc[5] + pc[6])
                for j in mine:
                    if r > 0:
                        # trivially satisfied (consumer progress implies the
                        # previous DMA on this sem completed); proves update
                        # ordering to the race detector
                        eng.wait_ge(sem_ld[j][b], 16 * r)
                    co = c0 % CHUNK
                    eng.dma_start(
                        out=ld[j][b][:, :cw],
                        in_=hs[f"{name}_s{j}"][c0 // CHUNK, :, co:co + cw],
                    ).then_inc(sem_ld[j][b], 16)
                # output stores lag 2 chunks so their compute-wait never
                # starves this stream's load lookahead
                if c >= 2 and _out_issuer(c - 2) == ename:
                    _store(c - 2)
            for x in range(max(C - 2, 0), C):
                if _out_issuer(x) == ename:
                    _store(x)
            for b in range(OBUF):
                if _out_issuer(b) != ename:
                    continue
                n_uses = len([1 for c in range(C) if c % OBUF == b])
                if n_uses:
                    eng.wait_ge(sem_ot[b], 16 * n_uses)

        @block.sync
        def _(sync):
            _stream(sync, 'sync')

        @block.scalar
        def _(scalar):
            _stream(scalar, 'scalar')

        @block.gpsimd
        def _(gpsimd):
            _stream(gpsimd, 'gpsimd')

        @block.tensor
        def _(tensor):
            for k, (ci, rc, j0, w, off) in enumerate(subs):
                (name, _, ns, c0, cw, ss, nsub) = chunks[ci]
                b = ci % NBUF
                r = ci // NBUF
                if k == 0:
                    tensor.wait_ge(sem_mats, 16)
                if j0 == 0:
                    for j in range(ns):
                        tensor.wait_ge(sem_ld[j][b], 16 * (r + 1))
                if k >= NBANKS:
                    tensor.wait_ge(sem_dve, k - NBANKS + 1)
                for j in range(ns):
                    mo = moffs[(name, j)]
                    mm = tensor.matmul(
                        ps[:rc, off:off + w],
                        lhsT=mats[:, mo:mo + rc],
                        rhs=ld[j][b][:, j0:j0 + w],
                        start=(j == 0),
                        stop=(j == ns - 1),
                        skip_group_check=True,
                    )
                mm.then_inc(sem_pe, 1)

        @block.vector
        def _(vector):
            for k, (ci, rc, j0, w, off) in enumerate(subs):
                (name, _, ns, c0, cw, ss, nsub) = chunks[ci]
                if j0 == 0 and ci >= OBUF:
                    # out slot free once its previous out-DMA completed
                    vector.wait_ge(sem_ot[ci % OBUF], 16 * (ci // OBUF))
                vector.wait_ge(sem_pe, k + 1)
                vector.tensor_copy(
                    out=ot[ci % OBUF][:rc, j0:j0 + w],
                    in_=ps[:rc, off:off + w],
                ).then_inc(sem_dve, 1)

    _NC_CACHE[spec_key] = nc
    return nc


# ---------------------------------------------------------------------------
# host-side orchestration
# ---------------------------------------------------------------------------

def _pad_up(n, m):
    return max(((n + m - 1) // m) * m, m)


def kernel(**inputs):
    from concourse.bass_utils import run_bass_kernel_spmd

    fe = np.asarray(inputs['feature_edge'], dtype=np.float32)
    fn = np.asarray(inputs['feature_node'], dtype=np.float32)
    S = np.asarray(inputs['S_hop'], dtype=np.float32)
    eih = np.asarray(inputs['edge_index_hop'])
    inv = np.asarray(inputs['edge_inverse']).astype(np.int64)
    atom_type = np.asarray(inputs['AtomType'])
    sel = {(0, 0): np.asarray(inputs['edge_sel_00']).astype(np.int64),
           (0, 1): np.asarray(inputs['edge_sel_01']).astype(np.int64),
           (1, 0): np.asarray(inputs['edge_sel_10']).astype(np.int64),
           (1, 1): np.asarray(inputs['edge_sel_11']).astype(np.int64)}
    cg = {(l1, l2): np.asarray(inputs[f'cg_{l1}_{l2}'], dtype=np.float32)
          for l1 in range(3) for l2 in range(3)}

    n1 = eih[0].astype(np.int64)
    n2 = eih[1].astype(np.int64)
    E = fe.shape[0]
    N = fn.shape[0]
    is_self = (n1 == n2) & (np.linalg.norm(S, axis=-1) < 1e-6)

    # g2b exactly as the reference computes it (sequential scatter, last wins)
    g2b = np.zeros(E, np.int64)
    for p in [(0, 0), (0, 1), (1, 0), (1, 1)]:
        g2b[sel[p]] = np.arange(len(sel[p]))

    # constant maps
    A = {(t1, t2): _build_A(cg, t1, t2) for t1 in range(2) for t2 in range(2)}
    B = {(t1, t2): A[(t2, t1)][:, _swap_cols(t1, t2)]
         for t1 in range(2) for t2 in range(2)}

    # feature-major copies for contiguous column gathers; fnz has a zero col at N
    feT = np.ascontiguousarray(fe.T)
    fnT = np.ascontiguousarray(fn.T)
    fnzT = np.concatenate([fnT, np.zeros((100, 1), np.float32)], axis=1)

    # ---- invariant check: structured (paired) fast path applicable? ----
    structured = _check_structured(E, inv, is_self, sel, atom_type, n1, n2)

    groups, writes, post_ham10 = _make_groups(
        structured, fe, fn, sel, inv, is_self, g2b, n1, A, B, E, N)
    # big-output groups first so the final store tail is small
    groups.sort(key=lambda g: -g['rc'])

    # ---- shard each group across cores, build in_maps ----
    spec_key = []
    meta = []
    for g in groups:
        n_items = len(g['streams'][0][1])
        per = -(-n_items // NUM_CORES)
        Cg = _pad_up(per, PAD)
        spec_key.append((g['name'], g['rc'], len(g['streams']), Cg))
        meta.append((g, n_items, per, Cg))
    spec_key = tuple(spec_key)

    moffs, mtot = _mat_layout(spec_key)
    mats_all = np.zeros((100, mtot), np.float32)
    for g in groups:
        for j in range(len(g['streams'])):
            off = moffs[(g['name'], j)]
            mats_all[:, off:off + g['rc']] = g['mats'][j]

    in_maps = [dict() for _ in range(NUM_CORES)]
    for c in range(NUM_CORES):
        in_maps[c]['mats_all'] = mats_all
    for (g, n_items, per, Cg) in meta:
        nch = -(-Cg // CHUNK)
        for j, (kind, idx) in enumerate(g['streams']):
            src = {'fe': feT, 'fn': fnT, 'fnz': fnzT}[kind]
            for c in range(NUM_CORES):
                lo = min(c * per, n_items)
                hi = min(lo + per, n_items)
                arr = np.zeros((100, nch * CHUNK), np.float32)
                if hi > lo:
                    arr[:, :hi - lo] = src[:, idx[lo:hi]]
                in_maps[c][f"{g['name']}_s{j}"] = np.ascontiguousarray(
                    arr.reshape(100, nch, CHUNK).transpose(1, 0, 2))

    nc = _build_nc(spec_key)
    global LAST_RESULT
    res = run_bass_kernel_spmd(nc, in_maps, core_ids=list(range(NUM_CORES)))
    LAST_RESULT = res
    results = res.results

    # ---- unshard: per group concat the per-core y slices ----
    y_of = {}
    for (g, n_items, per, Cg) in meta:
        parts = []
        for c in range(NUM_CORES):
            lo = min(c * per, n_items)
            hi = min(lo + per, n_items)
            if hi > lo:
                parts.append(np.asarray(results[c][f"{g['name']}_y"])[:, :hi - lo])
        y = np.concatenate(parts, axis=1) if parts else np.zeros((g['rc'], 0), np.float32)
        y_of[g['name']] = y.T          # [n_items, rc]

    # ---- assemble hams (index permutations only) ----
    shapes = [(len(sel[(0, 0)]), 4, 4), (len(sel[(0, 1)]), 4, 10),
              (len(sel[(1, 0)]), 10, 4), (len(sel[(1, 1)]), 10, 10)]
    hams = [np.zeros(s, np.float32) for s in shapes]
    for (gname, ham_idx, rows, transposed) in writes:
        r, c = shapes[ham_idx][1], shapes[ham_idx][2]
        y = y_of[gname]
        if transposed:
            blk = y.reshape(-1, c, r).transpose(0, 2, 1)
        else:
            blk = y.reshape(-1, r, c)
        hams[ham_idx][rows] = blk

    if post_ham10 is not None:
        # general path: ham10 = swap(ham01[eb12]) with jnp-style clamped gather
        eb12 = post_ham10
        hams[2] = hams[1][eb12].transpose(0, 2, 1)

    g2b_out = g2b.astype(np.int32)
    return (*hams, g2b_out)


def _check_structured(E, inv, is_self, sel, atom_type, n1, n2):
    if inv.min() < 0 or inv.max() >= E:
        return False
    if not np.array_equal(inv[inv], np.arange(E)):
        return False
    if not np.array_equal(inv == np.arange(E), is_self):
        return False
    allsel = np.concatenate([sel[p] for p in [(0, 0), (0, 1), (1, 0), (1, 1)]])
    if len(allsel) != E:
        return False
    if not np.array_equal(np.sort(allsel), np.arange(E)):
        return False
    t1 = atom_type[n1]
    t2 = atom_type[n2]
    for (a, b), s in sel.items():
        if not (np.all(t1[s] == a) and np.all(t2[s] == b)):
            return False
    # inverse edges swap the type pair
    if not (np.all(np.isin(inv[sel[(0, 1)]], sel[(1, 0)], assume_unique=True))):
        return False
    # self edges only on the diagonal (implied by type consistency + n1==n2)
    return True


def _make_groups(structured, fe, fn, sel, inv, is_self, g2b, n1, A, B, E, N):
    groups = []
    writes = []       # (group_name, ham_idx, row_indices, transposed)
    post_ham10 = None

    if structured:
        # off-diagonal: one item per sel01 edge; partner row of ham10 is its swap
        a = sel[(0, 1)]
        b = inv[a]
        groups.append(dict(name='p01', rc=40,
                           mats=[0.5 * A[(0, 1)], 0.5 * B[(0, 1)]],
                           streams=[('fe', a), ('fe', b)]))
        writes.append(('p01', 1, np.arange(len(a)), False))
        writes.append(('p01', 2, g2b[b], True))

        for t in (0, 1):
            s_tt = sel[(t, t)]
            hi = t * 2 + t      # 0 or 3
            rc = ORBITAL_SUM[t] ** 2
            selfm = is_self[s_tt]
            reg = s_tt[~selfm]
            can = reg[reg < inv[reg]]
            bp = inv[can]
            if len(can):
                groups.append(dict(name=f'p{t}{t}', rc=rc,
                                   mats=[0.5 * A[(t, t)], 0.5 * B[(t, t)]],
                                   streams=[('fe', can), ('fe', bp)]))
                writes.append((f'p{t}{t}', hi, g2b[can], False))
                writes.append((f'p{t}{t}', hi, g2b[bp], True))
            se = s_tt[selfm]
            if len(se):
                Msym = 0.5 * (A[(t, t)] + B[(t, t)])
                groups.append(dict(name=f's{t}', rc=rc,
                                   mats=[Msym, Msym],
                                   streams=[('fe', se), ('fn', n1[se])]))
                writes.append((f's{t}', hi, g2b[se], False))
    else:
        # general path: emulate the reference's clamped gathers row by row
        def partner(s12, p21):
            s21 = sel[p21]
            j = np.clip(g2b[np.clip(inv[s12], 0, E - 1)], 0, max(len(s21) - 1, 0))
            return s21[j] if len(s21) else np.zeros(len(s12), np.int64)

        for (pair, hi) in [((0, 0), 0), ((0, 1), 1), ((1, 1), 3)]:
            t1t, t2t = pair
            s12 = sel[pair]
            if not len(s12):
                continue
            b = partner(s12, (t2t, t1t))
            plain = ~is_self[s12] & ~is_self[b]
            rc = ORBITAL_SUM[t1t] * ORBITAL_SUM[t2t]
            nm = f'g{t1t}{t2t}'
            idx_p = np.nonzero(plain)[0]
            idx_m = np.nonzero(~plain)[0]
            if len(idx_p):
                groups.append(dict(name=nm, rc=rc,
                                   mats=[0.5 * A[pair], 0.5 * B[pair]],
                                   streams=[('fe', s12[idx_p]), ('fe', b[idx_p])]))
                writes.append((nm, hi, idx_p, False))
            if len(idx_m):
                am, bm = s12[idx_m], b[idx_m]
                fa = np.where(is_self[am], n1[am], N)
                fb = np.where(is_self[bm], n1[bm], N)
                groups.append(dict(name=nm + 'x', rc=rc,
                                   mats=[0.5 * A[pair], 0.5 * A[pair],
                                         0.5 * B[pair], 0.5 * B[pair]],
                                   streams=[('fe', am), ('fnz', fa),
                                            ('fe', bm), ('fnz', fb)]))
                writes.append((nm + 'x', hi, idx_m, False))
        s10 = sel[(1, 0)]
        n01 = len(sel[(0, 1)])
        post_ham10 = np.clip(g2b[np.clip(inv[s10], 0, E - 1)], 0, max(n01 - 1, 0))

    return groups, writes, post_ham10


if __name__ == '__main__':
    pass
